# revision 12
# baseline (speedup 1.0000x reference)
"""Trainium2 Bass kernel for MoE-with-LoRA-experts (nn_MoE_64098091925598).

Reference computation (N=8192 tokens, D=1024, E=8 experts, R=16, top-2):
    logits  = x @ W_gate.T                      [N, E]
    combine = scatter(softmax(top2(logits)))    [N, E] (2 nonzeros/row)
    moe     = sum_e combine[:,e] * (x @ A_e @ B_e)
    out     = moe + x @ W_base.T + b_base
Data-parallel over tokens across 8 NeuronCores (1024 tokens/core); every
core computes all 8 LoRA experts densely and masks by combine weights.

v5 notes (baseline 76.4us):
- The DMA engines ramp: the first transfers move at ~50 GB/s and reach
  full rate only ~8us later, so the schedule is dominated by how few
  bytes the front-end needs.  x is shipped ONLY as a bf16 + bf16
  residual pair (2+2 MiB instead of 4 MiB fp32 + 2 MiB bf16):
    logits = xb@[wgb|wgr] (16-wide stacked lhs) + xr@wgb
  which carries ~1e-6 absolute logits error (vs ~1e-4 for fp32r) --
  top-2 selection matches the fp32 reference.
- PE HAM clock gate: idle re-throttles the PE to 1.2GHz for ~3.4us, so
  filler matmuls bridge the known DMA-wait points and the matmul stream
  is ordered: gating-w0 (paced by xb0 arrival) -> LoRA-w0 -> base j0-3
  dt0 -> full front-end w1 -> base j0-3 dt1 -> j4-7.
- All matmuls bf16 except the tiny combine-expand; PSUM stays fp32.
- Output stored bf16 as [64,1024] halves on both rings, upcast on host.
"""

import numpy as np
import ml_dtypes

import concourse.mybir as mybir
import concourse.tile as tile
from concourse import bacc
from concourse.bass_utils import run_bass_kernel_spmd
from concourse.masks import make_identity

N_TOK, D, E, R, TOPK = 8192, 1024, 8, 16, 2
CORES = 8
NS = N_TOK // CORES  # tokens per core
ER = E * R  # 128, stacked expert-rank dim
DC = D // 128  # 8 contraction chunks
NJ = NS // 128  # 8 token chunks per core
NT = NS // 512  # 2 wide token tiles (waves) per core
JT = NJ // NT  # 4 token chunks per wave
DT = D // 512  # 2 dout tiles

f32 = mybir.dt.float32
f32r = mybir.dt.float32r
bf16 = mybir.dt.bfloat16

N_WARM = 10  # PE clock-gate warmers while first loads land
FILL_XA = 2  # fillers before the LoRA matmuls (a tile DMA margin)
FILL_WB = 2  # fillers before the first base matmul (W_base margin)

_CACHE: dict = {}


def _kernel_body(nc, tc, dram):
    xbT, xrT, wbT, a_p, b_p, wgp, exp_m, b_vec, out = dram
    xbT3 = xbT.rearrange("(c p) n -> p c n", p=128)
    xrT3 = xrT.rearrange("(c p) n -> p c n", p=128)
    wbT3 = wbT.rearrange("(c p) d -> p c d", p=128)

    from contextlib import ExitStack

    ctx = ExitStack()
    pw = ctx.enter_context(tc.tile_pool(name="weights", bufs=1))
    pg = ctx.enter_context(tc.tile_pool(name="gating", bufs=1))
    pmt = ctx.enter_context(tc.tile_pool(name="mmtmp", bufs=2))
    pout = ctx.enter_context(tc.tile_pool(name="outsb", bufs=4))
    ps_tp = ctx.enter_context(tc.tile_pool(name="ps_tp", bufs=2, space="PSUM"))
    ps_mm = ctx.enter_context(tc.tile_pool(name="ps_mm", bufs=2, space="PSUM"))
    ps_out = ctx.enter_context(tc.tile_pool(name="ps_out", bufs=3, space="PSUM"))
    ps_wm = ctx.enter_context(tc.tile_pool(name="ps_wm", bufs=1, space="PSUM"))

    # ---- PE prewarm: garbage matmuls, no data deps, never read -----
    warm_sb = pw.tile([128, 512], bf16, tag="warm")
    warm_ps = ps_wm.tile([128, 512], f32, tag="wm")
    nc.vector.memset(warm_sb, 0.0)

    def fill(n):
        for _ in range(n):
            nc.tensor.matmul(
                warm_ps, warm_sb[:, 0:128], warm_sb, start=True, stop=True
            )

    fill(N_WARM)

    # ---- Load phase: two HWDGE rings, FIFO per ring, arrival-ordered
    def ring(i):
        return nc.sync if i % 2 == 0 else nc.scalar

    # stacked gate weights [wgb | 0pad | wgr] bf16 (wgr lands at psum
    # partitions 32-39: psum reads must start at a multiple of 32),
    # plus a separate wgb copy for the residual pass
    WGW = 128
    wgs = pw.tile([128, DC, WGW], bf16, tag="wgs")
    nc.sync.dma_start(out=wgs, in_=wgp.rearrange("p (c e) -> p c e", e=WGW))
    wgb3 = pw.tile([128, DC, E], bf16, tag="wgb3")
    nc.scalar.dma_start(
        out=wgb3, in_=wgp.rearrange("p (c e) -> p c e", e=WGW)[:, :, 0:E]
    )
    exp_sb = pw.tile([E, ER], f32r, tag="expand")
    nc.scalar.dma_start(out=exp_sb, in_=exp_m)

    # x bf16 wave-0 (gating pass-1 + base/LoRA path)
    xb = [[None] * DC for _ in range(NT)]
    for c in range(DC):
        tl = pw.tile([128, 512], bf16, tag=f"xb0_{c}")
        ring(c).dma_start(out=tl, in_=xbT3[:, c, 0:512])
        xb[0][c] = tl

    a3 = pw.tile([128, DC, ER], bf16, tag="a")
    nc.sync.dma_start(out=a3, in_=a_p.rearrange("p (c r) -> p c r", r=ER))

    # x residual bf16 wave-0 (gating pass-2)
    xr = [[None] * DC for _ in range(NT)]
    for c in range(DC):
        tl = pw.tile([128, 512], bf16, tag=f"xr0_{c}")
        ring(c).dma_start(out=tl, in_=xrT3[:, c, 0:512])
        xr[0][c] = tl

    # W_base^T dout-half 0
    wb = [[None] * DC for _ in range(DT)]
    for c in range(DC):
        tl = pw.tile([128, 512], bf16, tag=f"wb0_{c}")
        ring(c).dma_start(out=tl, in_=wbT3[:, c, 0:512])
        wb[0][c] = tl
    b_sb = pw.tile([ER, D], bf16, tag="bflat")
    nc.scalar.dma_start(out=b_sb, in_=b_p)

    # wave-1: xb, xr, then W_base dout-half 1 -- all on the sync ring:
    # the scalar engine runs the gating sigmoids, and its NX must not
    # be stuck issuing DMA descriptor-gen when wave-1 gating arrives
    for c in range(DC):
        tl = pw.tile([128, 512], bf16, tag=f"xb1_{c}")
        nc.sync.dma_start(out=tl, in_=xbT3[:, c, 512:1024])
        xb[1][c] = tl
    for c in range(DC):
        tl = pw.tile([128, 512], bf16, tag=f"xr1_{c}")
        nc.sync.dma_start(out=tl, in_=xrT3[:, c, 512:1024])
        xr[1][c] = tl
    for c in range(DC):
        tl = pw.tile([128, 512], bf16, tag=f"wb1_{c}")
        nc.sync.dma_start(out=tl, in_=wbT3[:, c, 512:1024])
        wb[1][c] = tl

    bias_sb = pw.tile([128, D], f32, tag="bias")
    nc.gpsimd.dma_start(out=bias_sb, in_=b_vec.to_broadcast([128, D]))

    ident = pw.tile([128, 128], f32, tag="ident")
    make_identity(nc, ident)

    # ---- Front-end for one 512-token wave ---------------------------
    HT = [None] * NT  # H^T per wave [ER, 512] bf16

    def logits_p1(w):
        # [logits_b | pad | logits_r | pad]^T = [wgb|0|wgr|0]^T @ xb
        lgT_ps = ps_mm.tile([128, 512], f32, tag="mm")
        for c in range(DC):
            nc.tensor.matmul(
                lgT_ps, wgs[:, c, :], xb[w][c], start=(c == 0), stop=(c == DC - 1)
            )
        return lgT_ps

    def logits_p2(w):
        # wgb^T @ xr (residual correction pass), own PSUM bank
        lr_ps = ps_tp.tile([E, 512], f32, tag="tp", name=f"lr{w}")
        for c in range(DC):
            nc.tensor.matmul(
                lr_ps, wgb3[:, c, :], xr[w][c], start=(c == 0), stop=(c == DC - 1)
            )
        return lr_ps

    def gating(w, lgT_ps, lr_ps):
        # logits^T = (xb@wgb + xb@wgr) + xr@wgb
        hi_sb = pg.tile([E, 512], f32, tag=f"hi{w}")
        nc.vector.tensor_copy(hi_sb, lgT_ps[32 : 32 + E, :])
        t_sb = pg.tile([E, 512], f32, tag=f"t{w}")
        nc.vector.tensor_add(t_sb, lgT_ps[0:E, :], hi_sb)
        lgT_sb = pg.tile([E, 512], f32, tag=f"lgT{w}")
        nc.vector.tensor_add(lgT_sb, lr_ps, t_sb)

        # token-major logits chunks + sorted top-8 per token (PE transpose)
        lg3 = pg.tile([128, JT, E], f32, tag=f"lg3_{w}")
        mx = pg.tile([128, JT, E], f32, tag=f"mx{w}")
        for r in range(JT):
            tr_ps = ps_tp.tile([128, E], f32, tag="tp")
            nc.tensor.transpose(
                tr_ps, lgT_sb[:, r * 128 : (r + 1) * 128], ident[0:E, 0:E]
            )
            nc.vector.tensor_copy(lg3[:, r, :], tr_ps)
            nc.vector.max(out=mx[:, r, :], in_=lg3[:, r, :])

        # combine = 1{l==v1}*sigmoid(v1-v2) + 1{l==v2}*sigmoid(v2-v1)
        v1 = mx[:, :, 0:1]
        v2 = mx[:, :, 1:2]
        d21 = pg.tile([128, JT, 1], f32, tag=f"d21_{w}")
        nc.vector.tensor_sub(d21, v2, v1)
        w1 = pg.tile([128, JT, 1], f32, tag=f"w1_{w}")
        w2 = pg.tile([128, JT, 1], f32, tag=f"w2_{w}")
        nc.scalar.activation(w2, d21, mybir.ActivationFunctionType.Sigmoid)
        nc.scalar.activation(w1, d21, mybir.ActivationFunctionType.Sigmoid, scale=-1.0)

        eq1 = pg.tile([128, JT, E], f32, tag=f"eq1_{w}")
        eq2 = pg.tile([128, JT, E], f32, tag=f"eq2_{w}")
        cb = pg.tile([128, JT, E], f32, tag=f"cb{w}")
        bs = [128, JT, E]
        nc.vector.tensor_tensor(eq1, lg3, v1.to_broadcast(bs), mybir.AluOpType.is_equal)
        nc.vector.tensor_tensor(eq2, lg3, v2.to_broadcast(bs), mybir.AluOpType.is_equal)
        nc.vector.tensor_tensor(eq1, eq1, w1.to_broadcast(bs), mybir.AluOpType.mult)
        nc.vector.tensor_tensor(eq2, eq2, w2.to_broadcast(bs), mybir.AluOpType.mult)
        nc.vector.tensor_add(cb, eq1, eq2)
        return cb

    def lora_h(w):
        h_ps = ps_mm.tile([ER, 512], f32, tag="mm")
        for c in range(DC):
            nc.tensor.matmul(
                h_ps, a3[:, c, :], xb[w][c], start=(c == 0), stop=(c == DC - 1)
            )
        h_sb = pmt.tile([ER, 512], f32, tag="hsb")
        nc.vector.tensor_copy(h_sb, h_ps)
        return h_sb

    def combine_expand(w, cb, h_sb):
        cT_sb = pg.tile([E, 512], f32r, tag=f"cT{w}")
        for r in range(JT):
            cT_ps = ps_tp.tile([E, 128], f32, tag="tp")
            nc.tensor.transpose(cT_ps, cb[:, r, :], ident)
            nc.vector.tensor_copy(cT_sb[:, r * 128 : (r + 1) * 128], cT_ps)
        ce_ps = ps_mm.tile([ER, 512], f32, tag="mm")
        nc.tensor.matmul(ce_ps, exp_sb, cT_sb, start=True, stop=True)
        ht = pg.tile([ER, 512], bf16, tag=f"HT{w}")
        nc.vector.tensor_tensor(ht, ce_ps, h_sb, mybir.AluOpType.mult)
        HT[w] = ht

    # ---- Base+moe accumulation for one (dout half, token chunk) -----
    out_sb = [None] * NJ

    def chunk_pass(dt, j):
        w, jr = divmod(j, JT)
        jsl = slice(jr * 128, (jr + 1) * 128)
        dsl = slice(dt * 512, (dt + 1) * 512)
        if out_sb[j] is None:
            out_sb[j] = pout.tile([128, D], bf16, tag="osb", name=f"osb{j}")
        out_ps = ps_out.tile([128, 512], f32, tag="out")
        for c in range(DC):
            nc.tensor.matmul(
                out_ps, xb[w][c][:, jsl], wb[dt][c], start=(c == 0), stop=False
            )
        nc.tensor.matmul(out_ps, HT[w][:, jsl], b_sb[:, dsl], start=False, stop=True)
        nc.vector.tensor_add(out_sb[j][:, dsl], out_ps, bias_sb[:, dsl])
        if dt == DT - 1:
            j0 = j * 128
            nc.sync.dma_start(out=out[j0 : j0 + 64, :], in_=out_sb[j][0:64, :])
            nc.scalar.dma_start(out=out[j0 + 64 : j0 + 128, :], in_=out_sb[j][64:128, :])

    # ---- Program order (PE continuity is the whole game) ------------
    lg0 = logits_p1(0)
    fill(FILL_XA)
    h0 = lora_h(0)
    lr0 = logits_p2(0)
    cb0 = gating(0, lg0, lr0)  # vector math overlaps the next fills
    fill(FILL_WB)
    combine_expand(0, cb0, h0)
    for j in range(JT):
        chunk_pass(0, j)
    lg1 = logits_p1(1)
    h1 = lora_h(1)
    fill(4)
    lr1 = logits_p2(1)
    cb1 = gating(1, lg1, lr1)
    combine_expand(1, cb1, h1)
    for j in range(JT):
        chunk_pass(1, j)
    for j in range(JT, NJ):
        chunk_pass(0, j)
        chunk_pass(1, j)

    ctx.close()


def build_nc():
    nc = bacc.Bacc(
        "TRN2",
        target_bir_lowering=False,
        debug=False,
        enable_asserts=False,
        num_devices=CORES,
    )
    xbT = nc.dram_tensor("xbT", [D, NS], bf16, kind="ExternalInput").ap()
    xrT = nc.dram_tensor("xrT", [D, NS], bf16, kind="ExternalInput").ap()
    wbT = nc.dram_tensor("wbT", [D, D], bf16, kind="ExternalInput").ap()
    a_p = nc.dram_tensor("a_p", [128, DC * ER], bf16, kind="ExternalInput").ap()
    b_p = nc.dram_tensor("b_p", [ER, D], bf16, kind="ExternalInput").ap()
    wgp = nc.dram_tensor("wgp", [128, DC * 128], bf16, kind="ExternalInput").ap()
    exp_m = nc.dram_tensor("exp_m", [E, ER], f32, kind="ExternalInput").ap()
    b_vec = nc.dram_tensor("b_vec", [1, D], f32, kind="ExternalInput").ap()
    out = nc.dram_tensor("out", [NS, D], bf16, kind="ExternalOutput").ap()

    dram = (
        xbT,
        xrT,
        wbT,
        a_p,
        b_p,
        wgp,
        exp_m.bitcast(f32r),
        b_vec,
        out,
    )
    with tile.TileContext(nc) as tc:
        _kernel_body(nc, tc, dram)
    nc.compile()
    return nc


def host_prep(x, W_gate, A, B, W_base, b_base):
    """Shard + lay out the full inputs into 8 per-core input maps."""
    bft = ml_dtypes.bfloat16
    wbT = np.ascontiguousarray(W_base.T).astype(bft)  # [din, dout]
    a_p = np.ascontiguousarray(
        A.transpose(1, 0, 2).reshape(DC, 128, ER).transpose(1, 0, 2).reshape(128, -1)
    ).astype(bft)
    b_p = np.ascontiguousarray(B.reshape(ER, D)).astype(bft)
    # stacked [wgb | wgr] per chunk: [128, DC, 16] bf16
    wgT = W_gate.T.reshape(DC, 128, E).transpose(1, 0, 2)  # [p, c, e] fp32
    wgb = wgT.astype(bft)
    wgr = (wgT - wgb.astype(np.float32)).astype(bft)
    pad1 = np.zeros((128, DC, 32 - E), dtype=bft)
    pad2 = np.zeros((128, DC, 128 - 32 - E), dtype=bft)
    wgp = np.ascontiguousarray(
        np.concatenate([wgb, pad1, wgr, pad2], axis=2).reshape(128, -1)
    )
    exp_m = np.zeros((E, ER), dtype=np.float32)
    for e in range(E):
        exp_m[e, e * R : (e + 1) * R] = 1.0
    b_vec = np.ascontiguousarray(b_base.reshape(1, D))

    in_maps = []
    for cidx in range(CORES):
        xT = np.ascontiguousarray(x[cidx * NS : (cidx + 1) * NS].T)  # [D, NS]
        xbT = xT.astype(bft)
        xrT = (xT - xbT.astype(np.float32)).astype(bft)
        in_maps.append(
            {
                "xbT": xbT,
                "xrT": xrT,
                "wbT": wbT,
                "a_p": a_p,
                "b_p": b_p,
                "wgp": wgp,
                "exp_m": exp_m,
                "b_vec": b_vec,
            }
        )
    return in_maps


def kernel(x, W_gate, A, B, W_base, b_base):
    x = np.asarray(x, dtype=np.float32)
    W_gate = np.asarray(W_gate, dtype=np.float32)
    A = np.asarray(A, dtype=np.float32)
    B = np.asarray(B, dtype=np.float32)
    W_base = np.asarray(W_base, dtype=np.float32)
    b_base = np.asarray(b_base, dtype=np.float32)

    if "nc" not in _CACHE:
        _CACHE["nc"] = build_nc()
    nc = _CACHE["nc"]

    in_maps = host_prep(x, W_gate, A, B, W_base, b_base)
    res = run_bass_kernel_spmd(nc, in_maps, core_ids=list(range(CORES)))
    return np.concatenate(
        [np.asarray(res.results[c]["out"]).astype(np.float32) for c in range(CORES)],
        axis=0,
    )


# revision 13
# speedup vs baseline: 1.0518x; 1.0518x over previous
"""Trainium2 Bass kernel for MoE-with-LoRA-experts (nn_MoE_64098091925598).

Reference computation (N=8192 tokens, D=1024, E=8 experts, R=16, top-2):
    logits  = x @ W_gate.T                      [N, E]
    combine = scatter(softmax(top2(logits)))    [N, E] (2 nonzeros/row)
    moe     = sum_e combine[:,e] * (x @ A_e @ B_e)
    out     = moe + x @ W_base.T + b_base
Data-parallel over tokens across 8 NeuronCores (1024 tokens/core); every
core computes all 8 LoRA experts densely and masks by combine weights.

v5 notes (baseline 76.4us):
- The DMA engines ramp: the first transfers move at ~50 GB/s and reach
  full rate only ~8us later, so the schedule is dominated by how few
  bytes the front-end needs.  x is shipped ONLY as a bf16 + bf16
  residual pair (2+2 MiB instead of 4 MiB fp32 + 2 MiB bf16):
    logits = xb@[wgb|wgr] (16-wide stacked lhs) + xr@wgb
  which carries ~1e-6 absolute logits error (vs ~1e-4 for fp32r) --
  top-2 selection matches the fp32 reference.
- PE HAM clock gate: idle re-throttles the PE to 1.2GHz for ~3.4us, so
  filler matmuls bridge the known DMA-wait points and the matmul stream
  is ordered: gating-w0 (paced by xb0 arrival) -> LoRA-w0 -> base j0-3
  dt0 -> full front-end w1 -> base j0-3 dt1 -> j4-7.
- All matmuls bf16 except the tiny combine-expand; PSUM stays fp32.
- Output stored bf16 as [64,1024] halves on both rings, upcast on host.
"""

import numpy as np
import ml_dtypes

import concourse.mybir as mybir
import concourse.tile as tile
from concourse import bacc
from concourse.bass_utils import run_bass_kernel_spmd
from concourse.masks import make_identity

N_TOK, D, E, R, TOPK = 8192, 1024, 8, 16, 2
CORES = 8
NS = N_TOK // CORES  # tokens per core
ER = E * R  # 128, stacked expert-rank dim
DC = D // 128  # 8 contraction chunks
NJ = NS // 128  # 8 token chunks per core
NT = NS // 512  # 2 wide token tiles (waves) per core
JT = NJ // NT  # 4 token chunks per wave
DT = D // 512  # 2 dout tiles

f32 = mybir.dt.float32
f32r = mybir.dt.float32r
bf16 = mybir.dt.bfloat16

N_WARM = 10  # PE clock-gate warmers while first loads land
FILL_XA = 2  # fillers before the LoRA matmuls (a tile DMA margin)
FILL_WB = 2  # fillers before the first base matmul (W_base margin)

_CACHE: dict = {}


def _kernel_body(nc, tc, dram):
    xbT, xrT, wbT, a_p, b_p, wgp, exp_m, b_vec, out = dram
    xbT3 = xbT.rearrange("(c p) n -> p c n", p=128)
    xrT3 = xrT.rearrange("(c p) n -> p c n", p=128)
    wbT3 = wbT.rearrange("(c p) d -> p c d", p=128)

    from contextlib import ExitStack

    ctx = ExitStack()
    pw = ctx.enter_context(tc.tile_pool(name="weights", bufs=1))
    pg = ctx.enter_context(tc.tile_pool(name="gating", bufs=1))
    pmt = ctx.enter_context(tc.tile_pool(name="mmtmp", bufs=2))
    pout = ctx.enter_context(tc.tile_pool(name="outsb", bufs=4))
    ps_tp = ctx.enter_context(tc.tile_pool(name="ps_tp", bufs=2, space="PSUM"))
    ps_mm = ctx.enter_context(tc.tile_pool(name="ps_mm", bufs=2, space="PSUM"))
    ps_out = ctx.enter_context(tc.tile_pool(name="ps_out", bufs=3, space="PSUM"))
    ps_wm = ctx.enter_context(tc.tile_pool(name="ps_wm", bufs=1, space="PSUM"))

    # ---- PE prewarm: garbage matmuls, no data deps, never read -----
    warm_sb = pw.tile([128, 512], bf16, tag="warm")
    warm_ps = ps_wm.tile([128, 512], f32, tag="wm")
    nc.vector.memset(warm_sb, 0.0)

    def fill(n):
        for _ in range(n):
            nc.tensor.matmul(
                warm_ps, warm_sb[:, 0:128], warm_sb, start=True, stop=True
            )

    fill(N_WARM)

    # ---- Load phase: two HWDGE rings, FIFO per ring, arrival-ordered
    def ring(i):
        return nc.sync if i % 2 == 0 else nc.scalar

    # stacked gate weights [wgb | 0pad | wgr] bf16 (wgr lands at psum
    # partitions 32-39: psum reads must start at a multiple of 32),
    # plus a separate wgb copy for the residual pass
    WGW = 32 + E
    wgs = pw.tile([128, DC, WGW], bf16, tag="wgs")
    nc.sync.dma_start(out=wgs, in_=wgp.rearrange("p (c e) -> p c e", e=WGW))
    wgb3 = pw.tile([128, DC, E], bf16, tag="wgb3")
    nc.scalar.dma_start(
        out=wgb3, in_=wgp.rearrange("p (c e) -> p c e", e=WGW)[:, :, 0:E]
    )
    exp_sb = pw.tile([E, ER], f32r, tag="expand")
    nc.scalar.dma_start(out=exp_sb, in_=exp_m)

    # x bf16 wave-0 (gating pass-1 + base/LoRA path)
    xb = [[None] * DC for _ in range(NT)]
    for c in range(DC):
        tl = pw.tile([128, 512], bf16, tag=f"xb0_{c}")
        ring(c).dma_start(out=tl, in_=xbT3[:, c, 0:512])
        xb[0][c] = tl

    a3 = pw.tile([128, DC, ER], bf16, tag="a")
    nc.sync.dma_start(out=a3, in_=a_p.rearrange("p (c r) -> p c r", r=ER))

    # x residual bf16 wave-0 (gating pass-2)
    xr = [[None] * DC for _ in range(NT)]
    for c in range(DC):
        tl = pw.tile([128, 512], bf16, tag=f"xr0_{c}")
        ring(c).dma_start(out=tl, in_=xrT3[:, c, 0:512])
        xr[0][c] = tl

    # W_base^T dout-half 0
    wb = [[None] * DC for _ in range(DT)]
    for c in range(DC):
        tl = pw.tile([128, 512], bf16, tag=f"wb0_{c}")
        ring(c).dma_start(out=tl, in_=wbT3[:, c, 0:512])
        wb[0][c] = tl
    b_sb = pw.tile([ER, D], bf16, tag="bflat")
    nc.scalar.dma_start(out=b_sb, in_=b_p)

    # wave-1: xb, xr, then W_base dout-half 1 -- all on the sync ring:
    # the scalar engine runs the gating sigmoids, and its NX must not
    # be stuck issuing DMA descriptor-gen when wave-1 gating arrives
    for c in range(DC):
        tl = pw.tile([128, 512], bf16, tag=f"xb1_{c}")
        nc.sync.dma_start(out=tl, in_=xbT3[:, c, 512:1024])
        xb[1][c] = tl
    for c in range(DC):
        tl = pw.tile([128, 512], bf16, tag=f"xr1_{c}")
        nc.sync.dma_start(out=tl, in_=xrT3[:, c, 512:1024])
        xr[1][c] = tl
    for c in range(DC):
        tl = pw.tile([128, 512], bf16, tag=f"wb1_{c}")
        nc.sync.dma_start(out=tl, in_=wbT3[:, c, 512:1024])
        wb[1][c] = tl

    bias_sb = pw.tile([128, D], f32, tag="bias")
    nc.gpsimd.dma_start(out=bias_sb, in_=b_vec.to_broadcast([128, D]))

    ident = pw.tile([128, 128], f32, tag="ident")
    make_identity(nc, ident)

    # ---- Front-end for one 512-token wave ---------------------------
    HT = [None] * NT  # H^T per wave [ER, 512] bf16

    def logits_p1(w):
        # [logits_b | pad | logits_r]^T [40, 512] = [wgb|0|wgr]^T @ xb
        lgT_ps = ps_mm.tile([32 + E, 512], f32, tag="mm")
        for c in range(DC):
            nc.tensor.matmul(
                lgT_ps, wgs[:, c, :], xb[w][c], start=(c == 0), stop=(c == DC - 1),
                skip_group_check=True,
            )
            if w == 0 and c >= 3:
                fill(2)
        return lgT_ps

    def logits_p2(w):
        # wgb^T @ xr (residual correction pass), own PSUM bank
        lr_ps = ps_tp.tile([E, 512], f32, tag="tp", name=f"lr{w}")
        for c in range(DC):
            nc.tensor.matmul(
                lr_ps, wgb3[:, c, :], xr[w][c], start=(c == 0), stop=(c == DC - 1)
            )
        return lr_ps

    def gating(w, lgT_ps, lr_ps):
        # logits^T = (xb@wgb + xb@wgr) + xr@wgb
        hi_sb = pg.tile([E, 512], f32, tag=f"hi{w}")
        nc.vector.tensor_copy(hi_sb, lgT_ps[32 : 32 + E, :])
        t_sb = pg.tile([E, 512], f32, tag=f"t{w}")
        nc.vector.tensor_add(t_sb, lgT_ps[0:E, :], hi_sb)
        lgT_sb = pg.tile([E, 512], f32, tag=f"lgT{w}")
        nc.vector.tensor_add(lgT_sb, lr_ps, t_sb)

        # token-major logits chunks + sorted top-8 per token (PE transpose)
        lg3 = pg.tile([128, JT, E], f32, tag=f"lg3_{w}")
        mx = pg.tile([128, JT, E], f32, tag=f"mx{w}")
        for r in range(JT):
            tr_ps = ps_tp.tile([128, E], f32, tag="tp")
            nc.tensor.transpose(
                tr_ps, lgT_sb[:, r * 128 : (r + 1) * 128], ident[0:E, 0:E]
            )
            nc.vector.tensor_copy(lg3[:, r, :], tr_ps)
            nc.vector.max(out=mx[:, r, :], in_=lg3[:, r, :])

        # combine = 1{l==v1}*sigmoid(v1-v2) + 1{l==v2}*sigmoid(v2-v1)
        v1 = mx[:, :, 0:1]
        v2 = mx[:, :, 1:2]
        d21 = pg.tile([128, JT, 1], f32, tag=f"d21_{w}")
        nc.vector.tensor_sub(d21, v2, v1)
        w1 = pg.tile([128, JT, 1], f32, tag=f"w1_{w}")
        w2 = pg.tile([128, JT, 1], f32, tag=f"w2_{w}")
        nc.scalar.activation(w2, d21, mybir.ActivationFunctionType.Sigmoid)
        nc.scalar.activation(w1, d21, mybir.ActivationFunctionType.Sigmoid, scale=-1.0)

        eq1 = pg.tile([128, JT, E], f32, tag=f"eq1_{w}")
        eq2 = pg.tile([128, JT, E], f32, tag=f"eq2_{w}")
        cb = pg.tile([128, JT, E], f32, tag=f"cb{w}")
        bs = [128, JT, E]
        nc.vector.tensor_tensor(eq1, lg3, v1.to_broadcast(bs), mybir.AluOpType.is_equal)
        nc.vector.tensor_tensor(eq2, lg3, v2.to_broadcast(bs), mybir.AluOpType.is_equal)
        nc.vector.tensor_tensor(eq1, eq1, w1.to_broadcast(bs), mybir.AluOpType.mult)
        nc.vector.tensor_tensor(eq2, eq2, w2.to_broadcast(bs), mybir.AluOpType.mult)
        nc.vector.tensor_add(cb, eq1, eq2)
        return cb

    def lora_h(w):
        h_ps = ps_mm.tile([ER, 512], f32, tag="mm")
        for c in range(DC):
            nc.tensor.matmul(
                h_ps, a3[:, c, :], xb[w][c], start=(c == 0), stop=(c == DC - 1)
            )
        h_sb = pmt.tile([ER, 512], f32, tag="hsb")
        nc.vector.tensor_copy(h_sb, h_ps)
        return h_sb

    def combine_expand(w, cb, h_sb):
        cT_sb = pg.tile([E, 512], f32r, tag=f"cT{w}")
        for r in range(JT):
            cT_ps = ps_tp.tile([E, 128], f32, tag="tp")
            nc.tensor.transpose(cT_ps, cb[:, r, :], ident)
            nc.vector.tensor_copy(cT_sb[:, r * 128 : (r + 1) * 128], cT_ps)
        ce_ps = ps_mm.tile([ER, 512], f32, tag="mm")
        nc.tensor.matmul(ce_ps, exp_sb, cT_sb, start=True, stop=True)
        ht = pg.tile([ER, 512], bf16, tag=f"HT{w}")
        nc.vector.tensor_tensor(ht, ce_ps, h_sb, mybir.AluOpType.mult)
        HT[w] = ht

    # ---- Base+moe accumulation for one (dout half, token chunk) -----
    out_sb = [None] * NJ

    def chunk_pass(dt, j):
        w, jr = divmod(j, JT)
        jsl = slice(jr * 128, (jr + 1) * 128)
        dsl = slice(dt * 512, (dt + 1) * 512)
        if out_sb[j] is None:
            out_sb[j] = pout.tile([128, D], bf16, tag="osb", name=f"osb{j}")
        out_ps = ps_out.tile([128, 512], f32, tag="out")
        for c in range(DC):
            nc.tensor.matmul(
                out_ps, xb[w][c][:, jsl], wb[dt][c], start=(c == 0), stop=False
            )
        nc.tensor.matmul(out_ps, HT[w][:, jsl], b_sb[:, dsl], start=False, stop=True)
        nc.vector.tensor_add(out_sb[j][:, dsl], out_ps, bias_sb[:, dsl])
        if dt == DT - 1:
            j0 = j * 128
            nc.sync.dma_start(out=out[j0 : j0 + 64, :], in_=out_sb[j][0:64, :])
            nc.scalar.dma_start(out=out[j0 + 64 : j0 + 128, :], in_=out_sb[j][64:128, :])

    # ---- Program order (PE continuity is the whole game) ------------
    lg0 = logits_p1(0)
    fill(FILL_XA)
    h0 = lora_h(0)
    lr0 = logits_p2(0)
    cb0 = gating(0, lg0, lr0)  # vector math overlaps the next fills
    fill(FILL_WB)
    combine_expand(0, cb0, h0)
    for j in range(JT):
        chunk_pass(0, j)
    lg1 = logits_p1(1)
    h1 = lora_h(1)
    lr1 = logits_p2(1)
    cb1 = gating(1, lg1, lr1)
    combine_expand(1, cb1, h1)
    for j in range(JT):
        chunk_pass(1, j)
    for j in range(JT, NJ):
        chunk_pass(0, j)
        chunk_pass(1, j)

    ctx.close()


def build_nc():
    nc = bacc.Bacc(
        "TRN2",
        target_bir_lowering=False,
        debug=False,
        enable_asserts=False,
        num_devices=CORES,
    )
    xbT = nc.dram_tensor("xbT", [D, NS], bf16, kind="ExternalInput").ap()
    xrT = nc.dram_tensor("xrT", [D, NS], bf16, kind="ExternalInput").ap()
    wbT = nc.dram_tensor("wbT", [D, D], bf16, kind="ExternalInput").ap()
    a_p = nc.dram_tensor("a_p", [128, DC * ER], bf16, kind="ExternalInput").ap()
    b_p = nc.dram_tensor("b_p", [ER, D], bf16, kind="ExternalInput").ap()
    wgp = nc.dram_tensor("wgp", [128, DC * (32 + E)], bf16, kind="ExternalInput").ap()
    exp_m = nc.dram_tensor("exp_m", [E, ER], f32, kind="ExternalInput").ap()
    b_vec = nc.dram_tensor("b_vec", [1, D], f32, kind="ExternalInput").ap()
    out = nc.dram_tensor("out", [NS, D], bf16, kind="ExternalOutput").ap()

    dram = (
        xbT,
        xrT,
        wbT,
        a_p,
        b_p,
        wgp,
        exp_m.bitcast(f32r),
        b_vec,
        out,
    )
    with tile.TileContext(nc) as tc:
        _kernel_body(nc, tc, dram)
    nc.compile()
    return nc


def host_prep(x, W_gate, A, B, W_base, b_base):
    """Shard + lay out the full inputs into 8 per-core input maps."""
    bft = ml_dtypes.bfloat16
    wbT = np.ascontiguousarray(W_base.T).astype(bft)  # [din, dout]
    a_p = np.ascontiguousarray(
        A.transpose(1, 0, 2).reshape(DC, 128, ER).transpose(1, 0, 2).reshape(128, -1)
    ).astype(bft)
    b_p = np.ascontiguousarray(B.reshape(ER, D)).astype(bft)
    # stacked [wgb | wgr] per chunk: [128, DC, 16] bf16
    wgT = W_gate.T.reshape(DC, 128, E).transpose(1, 0, 2)  # [p, c, e] fp32
    wgb = wgT.astype(bft)
    wgr = (wgT - wgb.astype(np.float32)).astype(bft)
    pad = np.zeros((128, DC, 32 - E), dtype=bft)
    wgp = np.ascontiguousarray(
        np.concatenate([wgb, pad, wgr], axis=2).reshape(128, -1)
    )
    exp_m = np.zeros((E, ER), dtype=np.float32)
    for e in range(E):
        exp_m[e, e * R : (e + 1) * R] = 1.0
    b_vec = np.ascontiguousarray(b_base.reshape(1, D))

    in_maps = []
    for cidx in range(CORES):
        xT = np.ascontiguousarray(x[cidx * NS : (cidx + 1) * NS].T)  # [D, NS]
        xbT = xT.astype(bft)
        xrT = (xT - xbT.astype(np.float32)).astype(bft)
        in_maps.append(
            {
                "xbT": xbT,
                "xrT": xrT,
                "wbT": wbT,
                "a_p": a_p,
                "b_p": b_p,
                "wgp": wgp,
                "exp_m": exp_m,
                "b_vec": b_vec,
            }
        )
    return in_maps


def kernel(x, W_gate, A, B, W_base, b_base):
    x = np.asarray(x, dtype=np.float32)
    W_gate = np.asarray(W_gate, dtype=np.float32)
    A = np.asarray(A, dtype=np.float32)
    B = np.asarray(B, dtype=np.float32)
    W_base = np.asarray(W_base, dtype=np.float32)
    b_base = np.asarray(b_base, dtype=np.float32)

    if "nc" not in _CACHE:
        _CACHE["nc"] = build_nc()
    nc = _CACHE["nc"]

    in_maps = host_prep(x, W_gate, A, B, W_base, b_base)
    res = run_bass_kernel_spmd(nc, in_maps, core_ids=list(range(CORES)))
    return np.concatenate(
        [np.asarray(res.results[c]["out"]).astype(np.float32) for c in range(CORES)],
        axis=0,
    )
